# revision 8
# baseline (speedup 1.0000x reference)
"""Locally-connected layer (3x3, stride 1) on 8 Trainium2 NeuronCores.

Shapes (hardcoded):
  x      [B=32, C=96, H=32, W=32]  fp32
  weight [P=900, O=96, K=864]      fp32   (K = C*3*3, channel-major (c,kh,kw))
  bias   [P=900, O=96]             fp32
  out    [B=32, O=96, 30, 30]      fp32

Strategy:
  - Shard the 30x30 patch grid by output rows, padded to 32 rows -> 4 rows
    (120 patches) per core.  One SPMD program on all 8 cores.
  - The kernel is weight-DMA bound (each weight element is read once), so
    x/weight are cast to bf16 on the host and the output tile is stored
    bf16: ~20 MB of HBM traffic per core instead of ~39 MB fp32.
  - DMA bandwidth to SBUF is port-limited by partition coverage: a [96, F]
    tile reaches even AXI ports with 2x the bytes of odd ports (port id =
    ((p>>2)&7)<<1 | (p>>6)), capping at ~245-300 GB/s.  To balance ports,
    each chunk's (patch, tap) weight blocks are split 50/50:
      low  blocks: [96c, 96o] at partitions 0..95   -> 1 matmul  K=96 @ row 0
      high blocks: c0..63 at partitions 64..127 and c64..95 at partitions
                   32..63 -> 2 matmuls (K=64 @ row 64, K=32 @ row 32)
    Even ports then carry (64 + 32) rows per block pair and odd ports
    (32 + 64): exactly balanced -> ~356 GB/s (HBM-per-NC limit).
  - Per patch, contract K=864 as 9 taps of K=C=96 accumulating in PSUM:
    out[b,o] += x[:, i+di, j+dj, b].T @ W[p, dd][:, o].
    Stationary (lhsT) = x columns [c, 32b] read in place from SBUF-resident
    x slices laid out [c, h, w, b] (three partition-shifted copies: base 0,
    base 64 for c0..63, base 32 for c64..95).
  - Groups of <=4 adjacent patches are col-tiled onto the PE array via
    tile_position (x, 32u) so their matmuls run concurrently.
"""

import numpy as np

B, C, O, H, W = 32, 96, 96, 32, 32
OH = OW = 30
NCORES = 8
ROWS_PER_CORE = 4            # padded 32 output rows / 8 cores
P_CORE = ROWS_PER_CORE * OW  # 120 patches per core
XROWS = ROWS_PER_CORE + 2    # input rows needed per core (halo)
CH = 10                      # patches per weight chunk (cp*9 must be even)
NCHUNK = P_CORE // CH

LAST_RESULT = None           # BassKernelResults of the most recent run
_NC_CACHE = {}
KERNEL_KW = {}               # _build_bass kwargs for the kernel() path


def _chunk_groups(cp):
    """Split a chunk of cp consecutive patches into col-tile groups of <=4."""
    groups, j = [], 0
    while j < cp:
        g = min(4, cp - j)
        if cp - j == 5:      # avoid a trailing group of 1
            g = 3
        groups.append((j, g))
        j += g
    return groups


def _build_bass(reps=1, with_wdma=True, with_mm=True, with_out=True,
                row_out=False, chunk_patches=CH, wbufs=3, alt_ring=False,
                out_f32=False, out_ring="gpsimd", x_ring="gpsimd", obufs=1,
                out_cols=0):
    import concourse.bass as bass
    import concourse.mybir as mybir
    import concourse.tile as tile
    from concourse import bacc

    cp = chunk_patches
    assert OW % cp == 0 and (cp * 9) % 2 == 0
    cpr = OW // cp                       # chunks per row
    nb = cp * 9                          # weight blocks per chunk
    nh = nb // 2                         # low/high blocks per chunk
    nchunk = P_CORE // cp
    groups = _chunk_groups(cp)
    n_groups = ROWS_PER_CORE * cpr * len(groups)
    otw = n_groups * O

    f32 = mybir.dt.float32
    bf16 = mybir.dt.bfloat16
    out_dt = f32 if out_f32 else bf16
    nc = bacc.Bacc("TRN2", target_bir_lowering=False, debug=False,
                   num_devices=NCORES)
    xsd = nc.dram_tensor("xs", [C, XROWS, W, B], bf16, kind="ExternalInput")
    wld = nc.dram_tensor("wlo", [96, nchunk, nh * O], bf16, kind="ExternalInput")
    wad = nc.dram_tensor("wha", [64, nchunk, nh * O], bf16, kind="ExternalInput")
    wbd = nc.dram_tensor("whb", [32, nchunk, nh * O], bf16, kind="ExternalInput")
    od = nc.dram_tensor("out", [128, otw], out_dt, kind="ExternalOutput")

    with tile.TileContext(nc) as tc:
        with (
            tc.tile_pool(name="xp", bufs=1) as xp,
            tc.tile_pool(name="wp", bufs=wbufs) as wp,
            tc.tile_pool(name="op", bufs=obufs) as op,
            tc.tile_pool(name="pp", bufs=8, space=bass.MemorySpace.PSUM) as pp,
        ):
            xeng = getattr(nc, x_ring)
            xt0 = xp.tile([C, XROWS, W, B], bf16)
            xtA = xp.tile([128, XROWS, W, B], bf16)
            xtB = xp.tile([64, XROWS, W, B], bf16)
            xeng.dma_start(xt0[:], xsd[:])
            xeng.dma_start(xtA[64:128], xsd[0:64])
            xeng.dma_start(xtB[32:64], xsd[64:96])
            oeng = getattr(nc, out_ring)

            wfix = None
            if not with_wdma:
                wfix = (xp.tile([96, nh * O], bf16),
                        xp.tile([128, nh * O], bf16),
                        xp.tile([64, nh * O], bf16))
                nc.sync.dma_start(wfix[0][:], wld[:, 0])
                nc.sync.dma_start(wfix[1][64:128], wad[:, 0])
                nc.sync.dma_start(wfix[2][32:64], wbd[:, 0])

            for _rep in range(reps):
                ot = op.tile([128, otw], out_dt)
                if not with_mm and with_out:
                    nc.vector.memset(ot[:, 0:1], 0.0)
                for ch in range(ROWS_PER_CORE * cpr):
                    li, ci = ch // cpr, ch % cpr
                    if with_wdma:
                        wtL = wp.tile([96, nh * O], bf16)
                        wtA = wp.tile([128, nh * O], bf16)
                        wtB = wp.tile([64, nh * O], bf16)
                        ring = nc.scalar if (alt_ring and ch % 2) else nc.sync
                        ring.dma_start(wtL[:], wld[:, ch])
                        ring.dma_start(wtA[64:128], wad[:, ch])
                        ring.dma_start(wtB[32:64], wbd[:, ch])
                    else:
                        wtL, wtA, wtB = wfix
                    if with_mm:
                        for gi, (jo, gsz) in enumerate(groups):
                            j0 = ci * cp + jo
                            ps = pp.tile([128, O], f32)
                            for dd in range(9):
                                di, dj = dd // 3, dd % 3
                                for u in range(gsz):
                                    bi = (jo + u) * 9 + dd
                                    hw_ = slice((bi // 2) * O, (bi // 2 + 1) * O)
                                    pu = ps[32 * u:32 * (u + 1), :]
                                    h_, w_ = li + di, j0 + u + dj
                                    if bi % 2 == 0:
                                        nc.tensor.matmul(
                                            pu, xt0[:, h_, w_, :],
                                            wtL[:, hw_],
                                            start=(dd == 0), stop=(dd == 8),
                                            tile_position=(0, 32 * u))
                                    else:
                                        nc.tensor.matmul(
                                            pu, xtA[64:128, h_, w_, :],
                                            wtA[64:128, hw_],
                                            start=(dd == 0), stop=False,
                                            tile_position=(64, 32 * u))
                                        nc.tensor.matmul(
                                            pu, xtB[32:64, h_, w_, :],
                                            wtB[32:64, hw_],
                                            start=False, stop=(dd == 8),
                                            tile_position=(32, 32 * u))
                            g = (li * cpr + ci) * len(groups) + gi
                            nc.vector.tensor_copy(
                                ot[0:32 * gsz, g * O:(g + 1) * O],
                                ps[0:32 * gsz, :])
                    if with_out and row_out and ci == cpr - 1:
                        gw = cpr * len(groups) * O
                        oeng.dma_start(od[:, li * gw:(li + 1) * gw],
                                       ot[:, li * gw:(li + 1) * gw])
                if with_out and not row_out:
                    oc = out_cols or otw
                    oeng.dma_start(od[:, 0:oc], ot[:, 0:oc])
    nc.compile()
    return nc


def _build_tiny():
    """Same-I/O trivial kernel for marginal-cost benching."""
    import concourse.mybir as mybir
    import concourse.tile as tile
    from concourse import bacc

    cp = KERNEL_KW.get("chunk_patches", CH)
    cpr = OW // cp
    nh = cp * 9 // 2
    nchunk = P_CORE // cp
    n_groups = ROWS_PER_CORE * cpr * len(_chunk_groups(cp))
    otw = n_groups * O
    bf16 = mybir.dt.bfloat16
    out_dt = mybir.dt.float32 if KERNEL_KW.get("out_f32") else bf16
    nc = bacc.Bacc("TRN2", target_bir_lowering=False, debug=False,
                   num_devices=NCORES)
    xsd = nc.dram_tensor("xs", [C, XROWS, W, B], bf16, kind="ExternalInput")
    nc.dram_tensor("wlo", [96, nchunk, nh * O], bf16, kind="ExternalInput")
    nc.dram_tensor("wha", [64, nchunk, nh * O], bf16, kind="ExternalInput")
    nc.dram_tensor("whb", [32, nchunk, nh * O], bf16, kind="ExternalInput")
    od = nc.dram_tensor("out", [128, otw], out_dt, kind="ExternalOutput")
    with tile.TileContext(nc) as tc:
        with tc.tile_pool(name="tp", bufs=1) as tp:
            xt = tp.tile([C, B], bf16)
            nc.gpsimd.dma_start(xt[:], xsd[:, 0, 0, :])
            ot = tp.tile([128, 512], out_dt)
            nc.vector.memset(ot[:], 0.0)
            nc.gpsimd.dma_start(od[:, 0:512], ot[:])
    nc.compile()
    return nc


def _get_nc():
    key = tuple(sorted(KERNEL_KW.items()))
    if key not in _NC_CACHE:
        _NC_CACHE[key] = _build_bass(**KERNEL_KW)
    return _NC_CACHE[key]


def _prep_in_maps(x, weight):
    import ml_dtypes

    bf16 = ml_dtypes.bfloat16
    cp = KERNEL_KW.get("chunk_patches", CH)
    nh = cp * 9 // 2
    nchunk = P_CORE // cp
    # weight [900, O, C*3*3] -> [C, P_pad=960, dd, O]
    w5 = weight.reshape(OH * OW, O, C, 3, 3)
    wt = w5.transpose(2, 0, 3, 4, 1).reshape(C, OH * OW, 9, O)
    wpad = np.zeros((C, NCORES * P_CORE, 9, O), dtype=bf16)
    wpad[:, :OH * OW] = wt.astype(bf16)
    # -> per-core [C, nchunk, cp*9, O], split block-parity into lo/hi
    wpc = wpad.reshape(C, NCORES, nchunk, cp * 9, O)
    wlo = np.ascontiguousarray(
        wpc[:, :, :, 0::2].reshape(C, NCORES, nchunk, nh * O).transpose(1, 0, 2, 3))
    whi = wpc[:, :, :, 1::2].reshape(C, NCORES, nchunk, nh * O)
    wha = np.ascontiguousarray(whi[0:64].transpose(1, 0, 2, 3))
    whb = np.ascontiguousarray(whi[64:96].transpose(1, 0, 2, 3))

    # x [B, C, H, W] -> [C, H_pad=34, W, B]
    xt = x.transpose(1, 2, 3, 0)
    xpad = np.zeros((C, H + 2, W, B), dtype=bf16)
    xpad[:, :H] = xt.astype(bf16)

    in_maps = []
    for c in range(NCORES):
        in_maps.append({
            "xs": np.ascontiguousarray(
                xpad[:, ROWS_PER_CORE * c:ROWS_PER_CORE * c + XROWS]),
            "wlo": wlo[c],
            "wha": wha[c],
            "whb": whb[c],
        })
    return in_maps


def kernel(x, weight, bias):
    global LAST_RESULT
    from concourse.bass_utils import run_bass_kernel_spmd

    x = np.asarray(x, dtype=np.float32)
    weight = np.asarray(weight, dtype=np.float32)
    bias = np.asarray(bias, dtype=np.float32)

    in_maps = _prep_in_maps(x, weight)
    nc = _get_nc()
    LAST_RESULT = run_bass_kernel_spmd(
        nc, in_maps, core_ids=list(range(NCORES)), trace=False)

    # ---- gather: per-core [128, n_groups*96] -> full [B, O, 30, 30] ----
    cp = KERNEL_KW.get("chunk_patches", CH)
    groups = _chunk_groups(cp)
    cpr = OW // cp
    n_groups = ROWS_PER_CORE * cpr * len(groups)
    out = np.zeros((B, O, OH, OW), dtype=np.float32)
    for c in range(NCORES):
        oc = LAST_RESULT.results[c]["out"].astype(np.float32)
        oc = oc.reshape(4, 32, n_groups, O)
        for li in range(ROWS_PER_CORE):
            i = ROWS_PER_CORE * c + li
            if i >= OH:
                continue
            for ci in range(cpr):
                for gi, (jo, gsz) in enumerate(groups):
                    j0 = ci * cp + jo
                    g = (li * cpr + ci) * len(groups) + gi
                    blk = oc[:gsz, :, g, :]            # [u, b, o]
                    out[:, :, i, j0:j0 + gsz] = blk.transpose(1, 2, 0)
    out += bias.reshape(OH, OW, O).transpose(2, 0, 1)[None]
    return out


# revision 11
# speedup vs baseline: 2.8086x; 2.8086x over previous
"""Locally-connected layer (3x3, stride 1) on 8 Trainium2 NeuronCores.

Shapes (hardcoded):
  x      [B=32, C=96, H=32, W=32]  fp32
  weight [P=900, O=96, K=864]      fp32   (K = C*3*3, channel-major (c,kh,kw))
  bias   [P=900, O=96]             fp32
  out    [B=32, O=96, 30, 30]      fp32

Strategy:
  - Shard the 30x30 patch grid by output rows, padded to 32 rows -> 4 rows
    (120 patches) per core.  One SPMD program on all 8 cores.
  - The kernel is weight-DMA bound (each weight element is read once), so
    x/weight are cast to bf16 on the host and the output tile is stored
    bf16: ~20 MB of HBM traffic per core instead of ~39 MB fp32.
  - DMA bandwidth to SBUF is port-limited by partition coverage: a [96, F]
    tile reaches even AXI ports with 2x the bytes of odd ports (port id =
    ((p>>2)&7)<<1 | (p>>6)), capping at ~245-300 GB/s.  To balance ports,
    each chunk's (patch, tap) weight blocks are split 50/50:
      low  blocks: [96c, 96o] at partitions 0..95   -> 1 matmul  K=96 @ row 0
      high blocks: c0..63 at partitions 64..127 and c64..95 at partitions
                   32..63 -> 2 matmuls (K=64 @ row 64, K=32 @ row 32)
    Even ports then carry (64 + 32) rows per block pair and odd ports
    (32 + 64): exactly balanced -> ~356 GB/s (HBM-per-NC limit).
  - Per patch, contract K=864 as 9 taps of K=C=96 accumulating in PSUM:
    out[b,o] += x[:, i+di, j+dj, b].T @ W[p, dd][:, o].
    Stationary (lhsT) = x columns [c, 32b] read in place from SBUF-resident
    x slices laid out [c, h, w, b] (three partition-shifted copies: base 0,
    base 64 for c0..63, base 32 for c64..95).
  - Groups of <=4 adjacent patches are col-tiled onto the PE array via
    tile_position (x, 32u) so their matmuls run concurrently.
"""

import numpy as np

B, C, O, H, W = 32, 96, 96, 32, 32
OH = OW = 30
NCORES = 8
ROWS_PER_CORE = 4            # padded 32 output rows / 8 cores
P_CORE = ROWS_PER_CORE * OW  # 120 patches per core
XROWS = ROWS_PER_CORE + 2    # input rows needed per core (halo)
CH = 10                      # patches per weight chunk (cp*9 must be even)
NCHUNK = P_CORE // CH

LAST_RESULT = None           # BassKernelResults of the most recent run
_NC_CACHE = {}
KERNEL_KW = {}               # _build_bass kwargs for the kernel() path


def _chunk_groups(cp):
    """Split a chunk of cp consecutive patches into col-tile groups of <=4."""
    groups, j = [], 0
    while j < cp:
        g = min(4, cp - j)
        if cp - j == 5:      # avoid a trailing group of 1
            g = 3
        groups.append((j, g))
        j += g
    return groups


def _build_bass(reps=1, with_wdma=True, with_mm=True, with_out=True,
                row_out=False, chunk_patches=CH, wbufs=3, alt_ring=False,
                out_f32=False, out_ring="gpsimd", x_ring="gpsimd", obufs=1,
                out_cols=0, high_dds=(2, 6), w_rings=("sync", "scalar", "gpsimd")):
    import concourse.bass as bass
    import concourse.mybir as mybir
    import concourse.tile as tile
    from concourse import bacc

    cp = chunk_patches
    assert OW % cp == 0
    cpr = OW // cp                       # chunks per row
    low_dds = [d for d in range(9) if d not in high_dds]
    nlo, nhi = len(low_dds), len(high_dds)
    nchunk = P_CORE // cp
    groups = _chunk_groups(cp)
    n_groups = ROWS_PER_CORE * cpr * len(groups)
    otw = n_groups * O

    f32 = mybir.dt.float32
    bf16 = mybir.dt.bfloat16
    out_dt = f32 if out_f32 else bf16
    nc = bacc.Bacc("TRN2", target_bir_lowering=False, debug=False,
                   num_devices=NCORES)
    xsd = nc.dram_tensor("xs", [C, XROWS, W, B], bf16, kind="ExternalInput")
    wld = nc.dram_tensor("wlo", [96, nchunk, cp * nlo * O], bf16, kind="ExternalInput")
    wad = nc.dram_tensor("wha", [64, nchunk, max(cp * nhi, 1) * O], bf16, kind="ExternalInput")
    wbd = nc.dram_tensor("whb", [32, nchunk, max(cp * nhi, 1) * O], bf16, kind="ExternalInput")
    od = nc.dram_tensor("out", [128, otw], out_dt, kind="ExternalOutput")

    with tile.TileContext(nc) as tc:
        with (
            tc.tile_pool(name="xp", bufs=1) as xp,
            tc.tile_pool(name="wp", bufs=wbufs) as wp,
            tc.tile_pool(name="op", bufs=obufs) as op,
            tc.tile_pool(name="pp", bufs=8, space=bass.MemorySpace.PSUM) as pp,
        ):
            xeng = getattr(nc, x_ring)
            xt0 = xp.tile([C, XROWS, W, B], bf16)
            xtA = xp.tile([128, XROWS, W, B], bf16)
            xtB = xp.tile([64, XROWS, W, B], bf16)
            xeng.dma_start(xt0[:], xsd[:])
            xeng.dma_start(xtA[64:128], xsd[0:64])
            xeng.dma_start(xtB[32:64], xsd[64:96])
            oeng = getattr(nc, out_ring)

            wfix = None
            if not with_wdma:
                wfL = xp.tile([96, cp * nlo * O], bf16)
                wfA = xp.tile([128, max(cp * nhi, 1) * O], bf16)
                wfB = xp.tile([64, max(cp * nhi, 1) * O], bf16)
                wfix = (wfL, wfA, wfB)
                nc.sync.dma_start(wfL[:], wld[:, 0])
                if nhi:
                    nc.sync.dma_start(wfA[64:128], wad[:, 0])
                    nc.sync.dma_start(wfB[32:64], wbd[:, 0])

            for _rep in range(reps):
                ot = op.tile([128, otw], out_dt)
                if not with_mm and with_out:
                    nc.vector.memset(ot[:, 0:1], 0.0)
                for ch in range(ROWS_PER_CORE * cpr):
                    li, ci = ch // cpr, ch % cpr
                    if with_wdma:
                        wtL = wp.tile([96, cp * nlo * O], bf16)
                        wtA = wp.tile([128, max(cp * nhi, 1) * O], bf16)
                        wtB = wp.tile([64, max(cp * nhi, 1) * O], bf16)
                        rL, rA, rB = (getattr(nc, r) for r in w_rings)
                        rL.dma_start(wtL[:], wld[:, ch])
                        if nhi:
                            rA.dma_start(wtA[64:128], wad[:, ch])
                            rB.dma_start(wtB[32:64], wbd[:, ch])
                    else:
                        wtL, wtA, wtB = wfix
                    if with_mm:
                        # phase-ordered MM emission: all K=96 low matmuls of
                        # the chunk, then all K=64@64, then all K=32@32 -- the
                        # PE pays a pipeline flush when the tile config
                        # changes, so batch configs.
                        pss = []
                        for gi, (jo, gsz) in enumerate(groups):
                            ps = pp.tile([128, O], f32)
                            pss.append(ps)
                            j0 = ci * cp + jo
                            for ldi, dd in enumerate(low_dds):
                                di, dj = dd // 3, dd % 3
                                for u in range(gsz):
                                    nc.tensor.matmul(
                                        ps[32 * u:32 * (u + 1), :],
                                        xt0[:, li + di, j0 + u + dj, :],
                                        wtL[:, ((jo + u) * nlo + ldi) * O:
                                            ((jo + u) * nlo + ldi + 1) * O],
                                        start=(ldi == 0),
                                        stop=(ldi == nlo - 1 and nhi == 0),
                                        tile_position=(0, 32 * u))
                        for hp, (xh, wh, base) in enumerate(
                                ((xtA, wtA, 64), (xtB, wtB, 32))):
                            lo, hi = base, base + 64 if base == 64 else base + 32
                            hi = 128 if base == 64 else 64
                            for gi, (jo, gsz) in enumerate(groups):
                                ps = pss[gi]
                                j0 = ci * cp + jo
                                for hdi, dd in enumerate(high_dds):
                                    di, dj = dd // 3, dd % 3
                                    for u in range(gsz):
                                        nc.tensor.matmul(
                                            ps[32 * u:32 * (u + 1), :],
                                            xh[lo:hi, li + di, j0 + u + dj, :],
                                            wh[lo:hi, ((jo + u) * nhi + hdi) * O:
                                               ((jo + u) * nhi + hdi + 1) * O],
                                            start=False,
                                            stop=(hp == 1 and hdi == nhi - 1),
                                            tile_position=(base, 32 * u))
                        for gi, (jo, gsz) in enumerate(groups):
                            g = (li * cpr + ci) * len(groups) + gi
                            nc.vector.tensor_copy(
                                ot[0:32 * gsz, g * O:(g + 1) * O],
                                pss[gi][0:32 * gsz, :])
                    if with_out and row_out and ci == cpr - 1:
                        gw = cpr * len(groups) * O
                        oeng.dma_start(od[:, li * gw:(li + 1) * gw],
                                       ot[:, li * gw:(li + 1) * gw])
                if with_out and not row_out:
                    oc = out_cols or otw
                    oeng.dma_start(od[:, 0:oc], ot[:, 0:oc])
    nc.compile()
    return nc


def _build_tiny():
    """Same-I/O trivial kernel for marginal-cost benching."""
    import concourse.mybir as mybir
    import concourse.tile as tile
    from concourse import bacc

    cp = KERNEL_KW.get("chunk_patches", CH)
    cpr = OW // cp
    high_dds = KERNEL_KW.get("high_dds", (2, 6))
    nlo, nhi = 9 - len(high_dds), len(high_dds)
    nchunk = P_CORE // cp
    n_groups = ROWS_PER_CORE * cpr * len(_chunk_groups(cp))
    otw = n_groups * O
    bf16 = mybir.dt.bfloat16
    out_dt = mybir.dt.float32 if KERNEL_KW.get("out_f32") else bf16
    nc = bacc.Bacc("TRN2", target_bir_lowering=False, debug=False,
                   num_devices=NCORES)
    xsd = nc.dram_tensor("xs", [C, XROWS, W, B], bf16, kind="ExternalInput")
    nc.dram_tensor("wlo", [96, nchunk, cp * nlo * O], bf16, kind="ExternalInput")
    nc.dram_tensor("wha", [64, nchunk, max(cp * nhi, 1) * O], bf16, kind="ExternalInput")
    nc.dram_tensor("whb", [32, nchunk, max(cp * nhi, 1) * O], bf16, kind="ExternalInput")
    od = nc.dram_tensor("out", [128, otw], out_dt, kind="ExternalOutput")
    with tile.TileContext(nc) as tc:
        with tc.tile_pool(name="tp", bufs=1) as tp:
            xt = tp.tile([C, B], bf16)
            nc.gpsimd.dma_start(xt[:], xsd[:, 0, 0, :])
            ot = tp.tile([128, 512], out_dt)
            nc.vector.memset(ot[:], 0.0)
            nc.gpsimd.dma_start(od[:, 0:512], ot[:])
    nc.compile()
    return nc


def _get_nc():
    key = tuple(sorted(KERNEL_KW.items()))
    if key not in _NC_CACHE:
        _NC_CACHE[key] = _build_bass(**KERNEL_KW)
    return _NC_CACHE[key]


def _prep_in_maps(x, weight):
    import ml_dtypes

    bf16 = ml_dtypes.bfloat16
    cp = KERNEL_KW.get("chunk_patches", CH)
    high_dds = list(KERNEL_KW.get("high_dds", (2, 6)))
    low_dds = [d for d in range(9) if d not in high_dds]
    nlo, nhi = len(low_dds), len(high_dds)
    nchunk = P_CORE // cp
    # weight [900, O, C*3*3] -> [C, P_pad=960, dd, O]
    w5 = weight.reshape(OH * OW, O, C, 3, 3)
    wt = w5.transpose(2, 0, 3, 4, 1).reshape(C, OH * OW, 9, O)
    wpad = np.zeros((C, NCORES * P_CORE, 9, O), dtype=bf16)
    wpad[:, :OH * OW] = wt.astype(bf16)
    # -> per-core [C, nchunk, cp, 9, O], split taps into lo (K=96 @ base 0)
    # and hi (K=64 @ base 64 + K=32 @ base 32)
    wpc = wpad.reshape(C, NCORES, nchunk, cp, 9, O)
    wlo = np.ascontiguousarray(
        wpc[:, :, :, :, low_dds].reshape(C, NCORES, nchunk, cp * nlo * O)
        .transpose(1, 0, 2, 3))
    if nhi:
        whi = wpc[:, :, :, :, high_dds].reshape(C, NCORES, nchunk, cp * nhi * O)
    else:
        whi = np.zeros((C, NCORES, nchunk, O), dtype=bf16)
    wha = np.ascontiguousarray(whi[0:64].transpose(1, 0, 2, 3))
    whb = np.ascontiguousarray(whi[64:96].transpose(1, 0, 2, 3))

    # x [B, C, H, W] -> [C, H_pad=34, W, B]
    xt = x.transpose(1, 2, 3, 0)
    xpad = np.zeros((C, H + 2, W, B), dtype=bf16)
    xpad[:, :H] = xt.astype(bf16)

    in_maps = []
    for c in range(NCORES):
        in_maps.append({
            "xs": np.ascontiguousarray(
                xpad[:, ROWS_PER_CORE * c:ROWS_PER_CORE * c + XROWS]),
            "wlo": wlo[c],
            "wha": wha[c],
            "whb": whb[c],
        })
    return in_maps


def kernel(x, weight, bias):
    global LAST_RESULT
    from concourse.bass_utils import run_bass_kernel_spmd

    x = np.asarray(x, dtype=np.float32)
    weight = np.asarray(weight, dtype=np.float32)
    bias = np.asarray(bias, dtype=np.float32)

    in_maps = _prep_in_maps(x, weight)
    nc = _get_nc()
    LAST_RESULT = run_bass_kernel_spmd(
        nc, in_maps, core_ids=list(range(NCORES)), trace=False)

    # ---- gather: per-core [128, n_groups*96] -> full [B, O, 30, 30] ----
    cp = KERNEL_KW.get("chunk_patches", CH)
    groups = _chunk_groups(cp)
    cpr = OW // cp
    n_groups = ROWS_PER_CORE * cpr * len(groups)
    out = np.zeros((B, O, OH, OW), dtype=np.float32)
    for c in range(NCORES):
        oc = LAST_RESULT.results[c]["out"].astype(np.float32)
        oc = oc.reshape(4, 32, n_groups, O)
        for li in range(ROWS_PER_CORE):
            i = ROWS_PER_CORE * c + li
            if i >= OH:
                continue
            for ci in range(cpr):
                for gi, (jo, gsz) in enumerate(groups):
                    j0 = ci * cp + jo
                    g = (li * cpr + ci) * len(groups) + gi
                    blk = oc[:gsz, :, g, :]            # [u, b, o]
                    out[:, :, i, j0:j0 + gsz] = blk.transpose(1, 2, 0)
    out += bias.reshape(OH, OW, O).transpose(2, 0, 1)[None]
    return out


# revision 12
# speedup vs baseline: 14.1446x; 5.0362x over previous
"""Locally-connected layer (3x3, stride 1) on 8 Trainium2 NeuronCores.

Shapes (hardcoded):
  x      [B=32, C=96, H=32, W=32]  fp32
  weight [P=900, O=96, K=864]      fp32   (K = C*3*3, channel-major (c,kh,kw))
  bias   [P=900, O=96]             fp32
  out    [B=32, O=96, 30, 30]      fp32

Strategy:
  - Shard the 30x30 patch grid by output rows, padded to 32 rows -> 4 rows
    (120 patches) per core.  One SPMD program on all 8 cores.
  - The kernel is weight-DMA bound (each weight element is read once), so
    x/weight are cast to bf16 on the host and the output tile is stored
    bf16: ~20 MB of HBM traffic per core instead of ~39 MB fp32.
  - DMA bandwidth to SBUF is port-limited by partition coverage: a [96, F]
    tile reaches even AXI ports with 2x the bytes of odd ports (port id =
    ((p>>2)&7)<<1 | (p>>6)), capping at ~245-300 GB/s.  To balance ports,
    each chunk's (patch, tap) weight blocks are split 50/50:
      low  blocks: [96c, 96o] at partitions 0..95   -> 1 matmul  K=96 @ row 0
      high blocks: c0..63 at partitions 64..127 and c64..95 at partitions
                   32..63 -> 2 matmuls (K=64 @ row 64, K=32 @ row 32)
    Even ports then carry (64 + 32) rows per block pair and odd ports
    (32 + 64): exactly balanced -> ~356 GB/s (HBM-per-NC limit).
  - Per patch, contract K=864 as 9 taps of K=C=96 accumulating in PSUM:
    out[b,o] += x[:, i+di, j+dj, b].T @ W[p, dd][:, o].
    Stationary (lhsT) = x columns [c, 32b] read in place from SBUF-resident
    x slices laid out [c, h, w, b] (three partition-shifted copies: base 0,
    base 64 for c0..63, base 32 for c64..95).
  - Groups of <=4 adjacent patches are col-tiled onto the PE array via
    tile_position (x, 32u) so their matmuls run concurrently.
"""

import numpy as np

B, C, O, H, W = 32, 96, 96, 32, 32
OH = OW = 30
NCORES = 8
ROWS_PER_CORE = 4            # padded 32 output rows / 8 cores
P_CORE = ROWS_PER_CORE * OW  # 120 patches per core
XROWS = ROWS_PER_CORE + 2    # input rows needed per core (halo)
CH = 10                      # patches per weight chunk (cp*9 must be even)
NCHUNK = P_CORE // CH

LAST_RESULT = None           # BassKernelResults of the most recent run
_NC_CACHE = {}
KERNEL_KW = {}               # _build_bass kwargs for the kernel() path


def _chunk_groups(cp):
    """Split a chunk of cp consecutive patches into col-tile groups of <=4."""
    groups, j = [], 0
    while j < cp:
        g = min(4, cp - j)
        if cp - j == 5:      # avoid a trailing group of 1
            g = 3
        groups.append((j, g))
        j += g
    return groups


def _build_bass(reps=1, with_wdma=True, with_mm=True, with_out=True,
                row_out=False, chunk_patches=CH, wbufs=3, alt_ring=False,
                out_f32=False, out_ring="gpsimd", x_ring="gpsimd", obufs=1,
                out_cols=0, high_dds=(2, 6), w_rings=("sync", "scalar", "gpsimd")):
    import concourse.bass as bass
    import concourse.mybir as mybir
    import concourse.tile as tile
    from concourse import bacc

    cp = chunk_patches
    assert OW % cp == 0
    cpr = OW // cp                       # chunks per row
    low_dds = [d for d in range(9) if d not in high_dds]
    nlo, nhi = len(low_dds), len(high_dds)
    nchunk = P_CORE // cp
    groups = _chunk_groups(cp)
    n_groups = ROWS_PER_CORE * cpr * len(groups)
    otw = n_groups * O

    f32 = mybir.dt.float32
    bf16 = mybir.dt.bfloat16
    out_dt = f32 if out_f32 else bf16
    nc = bacc.Bacc("TRN2", target_bir_lowering=False, debug=False,
                   num_devices=NCORES)
    xsd = nc.dram_tensor("xs", [C, XROWS, W, B], bf16, kind="ExternalInput")
    wld = nc.dram_tensor("wlo", [96, nchunk, cp * nlo * O], bf16, kind="ExternalInput")
    whd = nc.dram_tensor("whab", [96, nchunk, max(cp * nhi, 1) * O], bf16, kind="ExternalInput")
    od = nc.dram_tensor("out", [128, otw], out_dt, kind="ExternalOutput")

    with tile.TileContext(nc) as tc:
        with (
            tc.tile_pool(name="xp", bufs=1) as xp,
            tc.tile_pool(name="wp", bufs=wbufs) as wp,
            tc.tile_pool(name="op", bufs=obufs) as op,
            tc.tile_pool(name="pp", bufs=8, space=bass.MemorySpace.PSUM) as pp,
        ):
            xeng = getattr(nc, x_ring)
            xt0 = xp.tile([C, XROWS, W, B], bf16)
            xtA = xp.tile([128, XROWS, W, B], bf16)
            xtB = xp.tile([64, XROWS, W, B], bf16)
            xeng.dma_start(xt0[:], xsd[:])
            xeng.dma_start(xtA[64:128], xsd[0:64])
            xeng.dma_start(xtB[32:64], xsd[64:96])
            oeng = getattr(nc, out_ring)

            wfix = None
            if not with_wdma:
                wfL = xp.tile([96, cp * nlo * O], bf16)
                wfH = xp.tile([128, max(cp * nhi, 1) * O], bf16)
                wfix = (wfL, wfH)
                nc.sync.dma_start(wfL[:], wld[:, 0])
                if nhi:
                    nc.sync.dma_start(wfH[32:128], whd[:, 0])

            for _rep in range(reps):
                ot = op.tile([128, otw], out_dt)
                if not with_mm and with_out:
                    nc.vector.memset(ot[:, 0:1], 0.0)
                for ch in range(ROWS_PER_CORE * cpr):
                    li, ci = ch // cpr, ch % cpr
                    if with_wdma:
                        wtL = wp.tile([96, cp * nlo * O], bf16)
                        wtH = wp.tile([128, max(cp * nhi, 1) * O], bf16)
                        rL, rH = (getattr(nc, r) for r in w_rings[:2])
                        rL.dma_start(wtL[:], wld[:, ch])
                        if nhi:
                            rH.dma_start(wtH[32:128], whd[:, ch])
                    else:
                        wtL, wtH = wfix
                    if with_mm:
                        # phase-ordered MM emission: all K=96 low matmuls of
                        # the chunk, then all K=64@64, then all K=32@32 -- the
                        # PE pays a pipeline flush when the tile config
                        # changes, so batch configs.
                        pss = []
                        for gi, (jo, gsz) in enumerate(groups):
                            ps = pp.tile([128, O], f32)
                            pss.append(ps)
                            j0 = ci * cp + jo
                            for ldi, dd in enumerate(low_dds):
                                di, dj = dd // 3, dd % 3
                                for u in range(gsz):
                                    nc.tensor.matmul(
                                        ps[32 * u:32 * (u + 1), :],
                                        xt0[:, li + di, j0 + u + dj, :],
                                        wtL[:, ((jo + u) * nlo + ldi) * O:
                                            ((jo + u) * nlo + ldi + 1) * O],
                                        start=(ldi == 0),
                                        stop=(ldi == nlo - 1 and nhi == 0),
                                        tile_position=(0, 32 * u))
                        for hp, (xh, wh, base) in enumerate(
                                ((xtA, wtH, 64), (xtB, wtH, 32))):
                            lo, hi = base, base + 64 if base == 64 else base + 32
                            hi = 128 if base == 64 else 64
                            for gi, (jo, gsz) in enumerate(groups):
                                ps = pss[gi]
                                j0 = ci * cp + jo
                                for hdi, dd in enumerate(high_dds):
                                    di, dj = dd // 3, dd % 3
                                    for u in range(gsz):
                                        nc.tensor.matmul(
                                            ps[32 * u:32 * (u + 1), :],
                                            xh[lo:hi, li + di, j0 + u + dj, :],
                                            wh[lo:hi, ((jo + u) * nhi + hdi) * O:
                                               ((jo + u) * nhi + hdi + 1) * O],
                                            start=False,
                                            stop=(hp == 1 and hdi == nhi - 1),
                                            tile_position=(base, 32 * u))
                        for gi, (jo, gsz) in enumerate(groups):
                            g = (li * cpr + ci) * len(groups) + gi
                            nc.vector.tensor_copy(
                                ot[0:32 * gsz, g * O:(g + 1) * O],
                                pss[gi][0:32 * gsz, :])
                    if with_out and row_out and ci == cpr - 1:
                        gw = cpr * len(groups) * O
                        oeng.dma_start(od[:, li * gw:(li + 1) * gw],
                                       ot[:, li * gw:(li + 1) * gw])
                if with_out and not row_out:
                    oc = out_cols or otw
                    oeng.dma_start(od[:, 0:oc], ot[:, 0:oc])
    nc.compile()
    return nc


def _build_tiny():
    """Same-I/O trivial kernel for marginal-cost benching."""
    import concourse.mybir as mybir
    import concourse.tile as tile
    from concourse import bacc

    cp = KERNEL_KW.get("chunk_patches", CH)
    cpr = OW // cp
    high_dds = KERNEL_KW.get("high_dds", (2, 6))
    nlo, nhi = 9 - len(high_dds), len(high_dds)
    nchunk = P_CORE // cp
    n_groups = ROWS_PER_CORE * cpr * len(_chunk_groups(cp))
    otw = n_groups * O
    bf16 = mybir.dt.bfloat16
    out_dt = mybir.dt.float32 if KERNEL_KW.get("out_f32") else bf16
    nc = bacc.Bacc("TRN2", target_bir_lowering=False, debug=False,
                   num_devices=NCORES)
    xsd = nc.dram_tensor("xs", [C, XROWS, W, B], bf16, kind="ExternalInput")
    nc.dram_tensor("wlo", [96, nchunk, cp * nlo * O], bf16, kind="ExternalInput")
    nc.dram_tensor("whab", [96, nchunk, max(cp * nhi, 1) * O], bf16, kind="ExternalInput")
    od = nc.dram_tensor("out", [128, otw], out_dt, kind="ExternalOutput")
    with tile.TileContext(nc) as tc:
        with tc.tile_pool(name="tp", bufs=1) as tp:
            xt = tp.tile([C, B], bf16)
            nc.gpsimd.dma_start(xt[:], xsd[:, 0, 0, :])
            ot = tp.tile([128, 512], out_dt)
            nc.vector.memset(ot[:], 0.0)
            nc.gpsimd.dma_start(od[:, 0:512], ot[:])
    nc.compile()
    return nc


def _get_nc():
    key = tuple(sorted(KERNEL_KW.items()))
    if key not in _NC_CACHE:
        _NC_CACHE[key] = _build_bass(**KERNEL_KW)
    return _NC_CACHE[key]


def _prep_in_maps(x, weight):
    import ml_dtypes

    bf16 = ml_dtypes.bfloat16
    cp = KERNEL_KW.get("chunk_patches", CH)
    high_dds = list(KERNEL_KW.get("high_dds", (2, 6)))
    low_dds = [d for d in range(9) if d not in high_dds]
    nlo, nhi = len(low_dds), len(high_dds)
    nchunk = P_CORE // cp
    # weight [900, O, C*3*3] -> [C, P_pad=960, dd, O]
    w5 = weight.reshape(OH * OW, O, C, 3, 3)
    wt = w5.transpose(2, 0, 3, 4, 1).reshape(C, OH * OW, 9, O)
    wpad = np.zeros((C, NCORES * P_CORE, 9, O), dtype=bf16)
    wpad[:, :OH * OW] = wt.astype(bf16)
    # -> per-core [C, nchunk, cp, 9, O], split taps into lo (K=96 @ base 0)
    # and hi (K=64 @ base 64 + K=32 @ base 32)
    wpc = wpad.reshape(C, NCORES, nchunk, cp, 9, O)
    wlo = np.ascontiguousarray(
        wpc[:, :, :, :, low_dds].reshape(C, NCORES, nchunk, cp * nlo * O)
        .transpose(1, 0, 2, 3))
    if nhi:
        whi = wpc[:, :, :, :, high_dds].reshape(C, NCORES, nchunk, cp * nhi * O)
    else:
        whi = np.zeros((C, NCORES, nchunk, O), dtype=bf16)
    # whab row r: r<32 -> c64..95 (dest partitions 32..63, K=32 matmul);
    #             r>=32 -> c0..63 (dest partitions 64..127, K=64 matmul)
    whab = np.ascontiguousarray(
        np.concatenate([whi[64:96], whi[0:64]], axis=0).transpose(1, 0, 2, 3))

    # x [B, C, H, W] -> [C, H_pad=34, W, B]
    xt = x.transpose(1, 2, 3, 0)
    xpad = np.zeros((C, H + 2, W, B), dtype=bf16)
    xpad[:, :H] = xt.astype(bf16)

    in_maps = []
    for c in range(NCORES):
        in_maps.append({
            "xs": np.ascontiguousarray(
                xpad[:, ROWS_PER_CORE * c:ROWS_PER_CORE * c + XROWS]),
            "wlo": wlo[c],
            "whab": whab[c],
        })
    return in_maps


def kernel(x, weight, bias):
    global LAST_RESULT
    from concourse.bass_utils import run_bass_kernel_spmd

    x = np.asarray(x, dtype=np.float32)
    weight = np.asarray(weight, dtype=np.float32)
    bias = np.asarray(bias, dtype=np.float32)

    in_maps = _prep_in_maps(x, weight)
    nc = _get_nc()
    LAST_RESULT = run_bass_kernel_spmd(
        nc, in_maps, core_ids=list(range(NCORES)), trace=False)

    # ---- gather: per-core [128, n_groups*96] -> full [B, O, 30, 30] ----
    cp = KERNEL_KW.get("chunk_patches", CH)
    groups = _chunk_groups(cp)
    cpr = OW // cp
    n_groups = ROWS_PER_CORE * cpr * len(groups)
    out = np.zeros((B, O, OH, OW), dtype=np.float32)
    for c in range(NCORES):
        oc = LAST_RESULT.results[c]["out"].astype(np.float32)
        oc = oc.reshape(4, 32, n_groups, O)
        for li in range(ROWS_PER_CORE):
            i = ROWS_PER_CORE * c + li
            if i >= OH:
                continue
            for ci in range(cpr):
                for gi, (jo, gsz) in enumerate(groups):
                    j0 = ci * cp + jo
                    g = (li * cpr + ci) * len(groups) + gi
                    blk = oc[:gsz, :, g, :]            # [u, b, o]
                    out[:, :, i, j0:j0 + gsz] = blk.transpose(1, 2, 0)
    out += bias.reshape(OH, OW, O).transpose(2, 0, 1)[None]
    return out
